# revision 29
# baseline (speedup 1.0000x reference)
"""Trainium2 Bass kernel for nn_DACA_29343216566277 (dual-GCN message passing).

Data-parallel over batch: 8 NeuronCores x 4 examples each. The full
per-example pipeline (LayerNorm -> attention-softmax adjacency -> 2-layer
GCN, two branches) runs on-device; the tiny [B,768]-level projection /
dense tail runs on host numpy.

Self-contained: hardcodes shapes/sharding; only imports the concourse
framework from the container's /opt install.
"""

import os
import sys
import time

import numpy as np

for _p in ("/opt/trn_rl_repo", "/root/.axon_site/_ro/trn_rl_repo"):
    if os.path.isdir(_p) and _p not in sys.path:
        sys.path.insert(0, _p)

import concourse.bass as bass
import concourse.bacc as bacc
import concourse.mybir as mybir
import concourse.tile as tile
from concourse.bass_utils import run_bass_kernel_spmd

AF = mybir.ActivationFunctionType
ALU = mybir.AluOpType
AX = mybir.AxisListType
F32 = mybir.dt.float32
PSUM = bass.MemorySpace.PSUM
DRAM = bass.MemorySpace.DRAM

B, S, D, H, MEM = 32, 512, 768, 8, 768
DK = D // H          # 96
NCORES = 8
BL = B // NCORES     # 4 examples per core
SC = S // 128        # 4 s-chunks
DC = D // 128        # 6 d-chunks
LN_EPS = 1e-6
INV_SQRT_DK = float(1.0 / np.sqrt(np.float32(DK)))

# PE dtype for matmuls: float32r streams 1 row/cycle (vs 4 for fp32) when the
# moving free dim is >=256; same 4-byte layout, relaxed multiply precision.
F32R = mybir.dt.float32r
USE_F32R = os.environ.get("KERNEL_MM_F32R", "1") == "1"
MMDT = F32R if USE_F32R else F32


def _r(ap):
    return ap

# consts tile column layout
C_BQ = {"fp": 0, "fc": 16}
C_BK = {"fp": 8, "fc": 24}
C_ASP = 32           # + ex*SC + sc : aspect-mask column chunks
C_RM = 48            # + ex*SC + sc : row (src) mask column chunks
C_ONE = 64
C_ZERO = 65
C_NCOL = 72


def _emit(nc, tc, dram, flags, ctx, n_iter=1):
    use_mask, ln_affine, gcn_bias, qk_bias = flags
    sb = ctx.enter_context(tc.tile_pool(name="sb", bufs=1))
    ps = ctx.enter_context(tc.tile_pool(name="ps", bufs=1, space=PSUM))
    dp = ctx.enter_context(tc.tile_pool(name="dp", bufs=1, space=DRAM))

    eye = sb.tile([128, 128], MMDT, name="eye_sb", tag="eye")
    nc.sync.dma_start(eye, dram["eye"].ap())
    omi = sb.tile([128, 128], F32, name="omi_sb", tag="omi")
    nc.vector.tensor_scalar(out=omi, in0=eye, scalar1=-1.0, scalar2=1.0,
                            op0=ALU.mult, op1=ALU.add)
    cst = sb.tile([128, C_NCOL], F32, name="cst_sb", tag="cst")
    nc.sync.dma_start(cst, dram["consts"].ap())
    colw = sb.tile([128, 1 + BL * SC], MMDT, name="colw_sb", tag="colw")
    nc.sync.dma_start(colw, dram["colw"].ap())
    zc = cst[:, C_ZERO:C_ZERO + 1]

    if ln_affine:
        abc = sb.tile([128, D], F32, name="abc_sb", tag="abc")
        nc.sync.dma_start(abc, dram["a_bc"].ap())
        bbc = sb.tile([128, D], F32, name="bbc_sb", tag="bbc")
        nc.sync.dma_start(bbc, dram["b_bc"].ap())
    if gcn_bias:
        gb = {}
        for br in ("fp", "fc"):
            for li in (0, 1):
                t = sb.tile([128, MEM], F32, name=f"b{li}bc_{br}", tag=f"b{li}bc_{br}")
                nc.sync.dma_start(t, dram[f"b{li}_bc_{br}"].ap())
                gb[(br, li)] = t
    if use_mask:
        cms = []
        for ex in range(BL):
            cmrow = sb.tile([1, S], F32, name=f"cmrow{ex}", tag="cmrow", bufs=2)
            nc.sync.dma_start(cmrow, dram["colmask"].ap()[ex:ex + 1, :])
            cm = sb.tile([128, S], F32, name=f"cm{ex}", tag=f"cm{ex}")
            nc.gpsimd.partition_broadcast(cm, cmrow)
            cms.append(cm)

    seq = dram["seq"].ap()

    if n_iter > 1:
        ctx.enter_context(tc.For_i(0, n_iter, 1))

    # ---- Phase 0: LayerNorm + transpose, spilled to DRAM ----
    xd = [dp.tile([128, SC, D], MMDT, name=f"xd{ex}", tag=f"xd{ex}")
          for ex in range(BL)]
    xTd = [dp.tile([128, DC, S], MMDT, name=f"xTd{ex}", tag=f"xTd{ex}")
           for ex in range(BL)]

    def ln_block(ex):
        xr = sb.tile([128, SC, D], MMDT, name=f"xr{ex}", tag="x", bufs=2)
        nc.sync.dma_start(xr, seq[ex].rearrange("(c p) d -> p c d", p=128))
        mvs = sb.tile([128, SC, 2], F32, name=f"mvs{ex}", tag="mvs", bufs=2)
        for sc in range(SC):
            bnst = sb.tile([128, 2, 6], F32, name=f"bnst{ex}_{sc}", tag="bnst", bufs=2)
            nc.vector.bn_stats(bnst[:, 0, :], xr[:, sc, 0:384])
            nc.vector.bn_stats(bnst[:, 1, :], xr[:, sc, 384:768])
            nc.vector.bn_aggr(mvs[:, sc, :], bnst)
        # rln = 1 / (sqrt(var * N/(N-1)) + eps); sqrt via exp(0.5*ln(v)),
        # batched over all 4 s-chunks so Ln/Exp table sets load once per ex
        lnv = sb.tile([128, SC], F32, name=f"lnv{ex}", tag="lnv", bufs=2)
        nc.scalar.activation(lnv, mvs[:, :, 1], AF.Ln, bias=zc, scale=float(D / (D - 1)))
        sd = sb.tile([128, SC], F32, name=f"sd{ex}", tag="sd", bufs=2)
        nc.scalar.activation(sd, lnv, AF.Exp, bias=zc, scale=0.5)
        nc.vector.tensor_scalar(out=sd, in0=sd, scalar1=LN_EPS, scalar2=None, op0=ALU.add)
        rln = sb.tile([128, SC], F32, name=f"rln{ex}", tag="rln", bufs=2)
        nc.vector.reciprocal(rln, sd)
        nm = sb.tile([128, SC], F32, name=f"nm{ex}", tag="nm", bufs=2)
        nc.vector.tensor_tensor(out=nm, in0=mvs[:, :, 0], in1=rln, op=ALU.mult)
        nc.vector.tensor_scalar(out=nm, in0=nm, scalar1=-1.0, scalar2=None, op0=ALU.mult)
        for sc in range(SC):
            nc.vector.tensor_scalar(out=xr[:, sc, :], in0=xr[:, sc, :],
                                    scalar1=rln[:, sc:sc + 1], scalar2=nm[:, sc:sc + 1],
                                    op0=ALU.mult, op1=ALU.add)
            if ln_affine:
                nc.vector.tensor_tensor(out=xr[:, sc, :], in0=xr[:, sc, :], in1=abc, op=ALU.mult)
                nc.vector.tensor_tensor(out=xr[:, sc, :], in0=xr[:, sc, :], in1=bbc, op=ALU.add)
        nc.sync.dma_start(xd[ex][:, :, :], xr[:, :, :])
        xT0 = sb.tile([128, DC, S], MMDT, name=f"xT0_{ex}", tag="xT", bufs=2)
        for dc in range(DC):
            tps = ps.tile([128, 512], MMDT, name=f"tp0_{ex}_{dc}", tag="ps", bufs=8)
            for sc in range(SC):
                nc.tensor.transpose(_r(tps[:, sc * 128:(sc + 1) * 128]),
                                    _r(xr[:, sc, dc * 128:(dc + 1) * 128]), _r(eye))
            nc.vector.tensor_copy(xT0[:, dc, :], tps)
        nc.sync.dma_start(xTd[ex][:, :, :], xT0[:, :, :])
        return xr, xT0

    # ---- Branches ----
    for br in ("fp", "fc"):
        ws = {}
        for wn in ("wq", "wk", "w0", "w1"):
            t = sb.tile([128, DC, D], MMDT, name=f"{wn}_{br}", tag=wn)
            nc.sync.dma_start(t, dram[f"{wn}T_{br}"].ap().rearrange("(c p) d -> p c d", p=128))
            ws[wn] = t
        for ex in range(BL):
            if br == "fp":
                x_sb, xT = ln_block(ex)
            else:
                x_sb = sb.tile([128, SC, D], MMDT, name=f"x_{br}{ex}", tag="x", bufs=2)
                nc.sync.dma_start(x_sb, xd[ex][:, :, :])
                xT = sb.tile([128, DC, S], MMDT, name=f"xT_{br}{ex}", tag="xT", bufs=2)
                nc.sync.dma_start(xT, xTd[ex][:, :, :])

            # --- scores / softmax / adjacency, head-major ---
            adj_ps = [ps.tile([128, 512], F32, name=f"adjps_{br}{ex}m{m}",
                              tag="ps", bufs=8) for m in range(SC)]
            for h in range(H):
                qph = ps.tile([96, 512], F32, name=f"qph_{br}{ex}h{h}", tag="ps", bufs=8)
                for kc in range(DC):
                    nc.tensor.matmul(qph, _r(ws["wq"][:, kc, DK * h:DK * (h + 1)]),
                                     _r(xT[:, kc, :]), start=(kc == 0), stop=(kc == DC - 1))
                qTh = sb.tile([96, 512], MMDT, name=f"qT_{br}{ex}h{h}", tag="qT", bufs=2)
                if qk_bias:
                    nc.vector.tensor_scalar(out=qTh, in0=qph,
                                            scalar1=cst[:96, C_BQ[br] + h:C_BQ[br] + h + 1],
                                            scalar2=None, op0=ALU.add)
                else:
                    nc.scalar.copy(qTh, qph)
                kph = ps.tile([96, 512], F32, name=f"kph_{br}{ex}h{h}", tag="ps", bufs=8)
                for kc in range(DC):
                    nc.tensor.matmul(kph, _r(ws["wk"][:, kc, DK * h:DK * (h + 1)]),
                                     _r(xT[:, kc, :]), start=(kc == 0), stop=(kc == DC - 1))
                kTh = sb.tile([96, 512], MMDT, name=f"kT_{br}{ex}h{h}", tag="kT", bufs=2)
                if qk_bias:
                    nc.vector.tensor_scalar(out=kTh, in0=kph,
                                            scalar1=cst[:96, C_BK[br] + h:C_BK[br] + h + 1],
                                            scalar2=None, op0=ALU.add)
                else:
                    nc.scalar.copy(kTh, kph)
                ehs = [sb.tile([128, 512], MMDT, name=f"e_{br}{ex}h{h}m{m}",
                               tag="e", bufs=4) for m in range(SC)]
                rs = sb.tile([128, SC], F32, name=f"rs_{br}{ex}h{h}", tag="rs", bufs=2)
                for m in range(SC):
                    sps = ps.tile([128, 512], F32, name=f"sps_{br}{ex}h{h}m{m}",
                                  tag="ps", bufs=8)
                    nc.tensor.matmul(sps, _r(qTh[:, m * 128:(m + 1) * 128]), _r(kTh[:, :]),
                                     start=True, stop=True)
                    if use_mask:
                        nc.scalar.activation(ehs[m], sps, AF.Exp, bias=zc,
                                             scale=INV_SQRT_DK)
                        nc.vector.tensor_tensor_reduce(
                            out=ehs[m], in0=ehs[m], in1=cms[ex], scale=1.0,
                            scalar=0.0, op0=ALU.mult, op1=ALU.add,
                            accum_out=rs[:, m:m + 1])
                    else:
                        nc.scalar.activation(ehs[m], sps, AF.Exp, bias=zc,
                                             scale=INV_SQRT_DK, accum_out=rs[:, m:m + 1])
                rra = sb.tile([128, SC], F32, name=f"rra_{br}{ex}h{h}", tag="rra", bufs=2)
                nc.vector.tensor_scalar(out=rra, in0=rs, scalar1=1e-30, scalar2=None,
                                        op0=ALU.add)
                rr = sb.tile([128, SC], F32, name=f"rr_{br}{ex}h{h}", tag="rr", bufs=2)
                nc.vector.reciprocal(rr, rra)
                for m in range(SC):
                    dg = sb.tile([128, 128], MMDT, name=f"dg_{br}{ex}h{h}m{m}",
                                 tag="dg", bufs=2)
                    nc.vector.tensor_scalar(out=dg, in0=eye, scalar1=rr[:, m:m + 1],
                                            scalar2=1.0 / H, op0=ALU.mult, op1=ALU.mult)
                    nc.tensor.matmul(adj_ps[m], _r(dg), _r(ehs[m]),
                                     start=(h == 0), stop=(h == H - 1),
                                     skip_group_check=True)

            adj_sb = sb.tile([128, SC, S], MMDT, name=f"adj_{br}{ex}", tag="adj")
            dsum = sb.tile([128, SC], F32, name=f"dsum_{br}{ex}", tag="dsum", bufs=2)
            for m in range(SC):
                if use_mask:
                    rmc = cst[:, C_RM + ex * SC + m:C_RM + ex * SC + m + 1]
                    nc.vector.tensor_scalar(out=adj_sb[:, m, :], in0=adj_ps[m],
                                            scalar1=rmc, scalar2=None, op0=ALU.mult)
                    me = sb.tile([128, 128], F32, name=f"me_{br}{ex}m{m}", tag="me", bufs=2)
                    nc.vector.tensor_scalar(out=me, in0=eye, scalar1=rmc, scalar2=None,
                                            op0=ALU.mult)
                    blk = adj_sb[:, m, m * 128:(m + 1) * 128]
                    nc.vector.tensor_tensor(out=blk, in0=blk, in1=omi, op=ALU.mult)
                    nc.vector.tensor_tensor(out=blk, in0=blk, in1=me, op=ALU.add)
                else:
                    nc.vector.tensor_copy(adj_sb[:, m, :], adj_ps[m])
                    blk = adj_sb[:, m, m * 128:(m + 1) * 128]
                    nc.vector.tensor_tensor(out=blk, in0=blk, in1=omi, op=ALU.mult)
                    nc.vector.tensor_tensor(out=blk, in0=blk, in1=eye, op=ALU.add)
                nc.vector.reduce_sum(out=dsum[:, m:m + 1], in_=adj_sb[:, m, :], axis=AX.X)
            dtm = sb.tile([128, SC], F32, name=f"dtm_{br}{ex}", tag="dtm", bufs=2)
            nc.vector.tensor_scalar(out=dtm, in0=dsum, scalar1=1.0, scalar2=None, op0=ALU.add)
            drp = sb.tile([128, SC], F32, name=f"drp_{br}{ex}", tag="drp", bufs=2)
            nc.vector.reciprocal(drp, dtm)

            adjT = sb.tile([128, SC, S], MMDT, name=f"adjT_{br}{ex}", tag="adjT")
            for b in range(SC):
                tps = ps.tile([128, 512], MMDT, name=f"tpa_{br}{ex}b{b}", tag="ps", bufs=8)
                for a in range(SC):
                    nc.tensor.transpose(_r(tps[:, a * 128:(a + 1) * 128]),
                                        _r(adj_sb[:, a, b * 128:(b + 1) * 128]), _r(eye))
                nc.vector.tensor_copy(adjT[:, b, :], tps)

            # --- GCN layer 1 ---
            t1 = sb.tile([128, DC, S], MMDT, name=f"t1_{br}{ex}", tag="tT")
            for dc in range(DC):
                tps = ps.tile([128, 512], F32, name=f"tl1_{br}{ex}d{dc}", tag="ps", bufs=8)
                for jc in range(SC):
                    nc.tensor.matmul(tps, _r(x_sb[:, jc, dc * 128:(dc + 1) * 128]),
                                     _r(adjT[:, jc, :]), start=(jc == 0), stop=(jc == SC - 1))
                nc.vector.tensor_copy(t1[:, dc, :], tps)
            o1 = sb.tile([128, SC, MEM], MMDT, name=f"o1_{br}{ex}", tag="o1")
            for sc in range(SC):
                up1 = ps.tile([128, 512], F32, name=f"u1a_{br}{ex}s{sc}", tag="ps", bufs=8)
                up2 = ps.tile([128, 256], F32, name=f"u1b_{br}{ex}s{sc}", tag="ps", bufs=8)
                for dc in range(DC):
                    lh = _r(t1[:, dc, sc * 128:(sc + 1) * 128])
                    nc.tensor.matmul(up1, lh, _r(ws["w0"][:, dc, 0:512]),
                                     start=(dc == 0), stop=(dc == DC - 1))
                    nc.tensor.matmul(up2, lh, _r(ws["w0"][:, dc, 512:MEM]),
                                     start=(dc == 0), stop=(dc == DC - 1))
                for up, c0, c1 in ((up1, 0, 512), (up2, 512, MEM)):
                    if gcn_bias:
                        nc.vector.tensor_tensor(out=o1[:, sc, c0:c1], in0=up,
                                                in1=gb[(br, 0)][:, c0:c1], op=ALU.add)
                        nc.vector.tensor_scalar(out=o1[:, sc, c0:c1], in0=o1[:, sc, c0:c1],
                                                scalar1=drp[:, sc:sc + 1], scalar2=0.0,
                                                op0=ALU.mult, op1=ALU.max)
                    else:
                        nc.vector.tensor_scalar(out=o1[:, sc, c0:c1], in0=up,
                                                scalar1=drp[:, sc:sc + 1], scalar2=0.0,
                                                op0=ALU.mult, op1=ALU.max)

            # --- GCN layer 2 ---
            t2 = sb.tile([128, DC, S], MMDT, name=f"t2_{br}{ex}", tag="tT")
            for mc in range(DC):
                tps = ps.tile([128, 512], F32, name=f"tl2_{br}{ex}d{mc}", tag="ps", bufs=8)
                for jc in range(SC):
                    nc.tensor.matmul(tps, _r(o1[:, jc, mc * 128:(mc + 1) * 128]),
                                     _r(adjT[:, jc, :]), start=(jc == 0), stop=(jc == SC - 1))
                nc.vector.tensor_copy(t2[:, mc, :], tps)
            o2 = sb.tile([128, SC, MEM], MMDT, name=f"o2_{br}{ex}", tag="o2")
            for sc in range(SC):
                up1 = ps.tile([128, 512], F32, name=f"u2a_{br}{ex}s{sc}", tag="ps", bufs=8)
                up2 = ps.tile([128, 256], F32, name=f"u2b_{br}{ex}s{sc}", tag="ps", bufs=8)
                for mc in range(DC):
                    lh = _r(t2[:, mc, sc * 128:(sc + 1) * 128])
                    nc.tensor.matmul(up1, lh, _r(ws["w1"][:, mc, 0:512]),
                                     start=(mc == 0), stop=(mc == DC - 1))
                    nc.tensor.matmul(up2, lh, _r(ws["w1"][:, mc, 512:MEM]),
                                     start=(mc == 0), stop=(mc == DC - 1))
                for up, c0, c1 in ((up1, 0, 512), (up2, 512, MEM)):
                    if gcn_bias:
                        nc.vector.tensor_tensor(out=o2[:, sc, c0:c1], in0=up,
                                                in1=gb[(br, 1)][:, c0:c1], op=ALU.add)
                        nc.vector.tensor_scalar(out=o2[:, sc, c0:c1], in0=o2[:, sc, c0:c1],
                                                scalar1=drp[:, sc:sc + 1], scalar2=0.0,
                                                op0=ALU.mult, op1=ALU.max)
                    else:
                        nc.vector.tensor_scalar(out=o2[:, sc, c0:c1], in0=up,
                                                scalar1=drp[:, sc:sc + 1], scalar2=0.0,
                                                op0=ALU.mult, op1=ALU.max)

            # --- weighted column-sum over S ---
            cs1 = ps.tile([1, 512], F32, name=f"cs1_{br}{ex}", tag="ps", bufs=8)
            cs2 = ps.tile([1, 256], F32, name=f"cs2_{br}{ex}", tag="ps", bufs=8)
            for sc in range(SC):
                if br == "fp":
                    colv = colw[:, 0:1]
                else:
                    colv = colw[:, 1 + ex * SC + sc:2 + ex * SC + sc]
                nc.tensor.matmul(cs1[0:1, :], _r(colv), _r(o2[:, sc, 0:512]),
                                 start=(sc == 0), stop=(sc == SC - 1),
                                 skip_group_check=True)
                nc.tensor.matmul(cs2[0:1, :], _r(colv), _r(o2[:, sc, 512:MEM]),
                                 start=(sc == 0), stop=(sc == SC - 1),
                                 skip_group_check=True)
            stg = sb.tile([1, MEM], F32, name=f"stg_{br}{ex}", tag="stg", bufs=1)
            nc.vector.tensor_copy(stg[:, 0:512], cs1)
            nc.vector.tensor_copy(stg[:, 512:MEM], cs2)
            nc.sync.dma_start(dram[f"out_{br}"].ap()[ex:ex + 1, :], stg)


def _build(flags, n_iter=1):
    use_mask, ln_affine, gcn_bias, qk_bias = flags
    nc = bacc.Bacc("TRN2", target_bir_lowering=False, debug=False,
                   num_devices=NCORES)
    dram = {
        "seq": nc.dram_tensor("seq", [BL, S, D], MMDT, kind="ExternalInput"),
        "eye": nc.dram_tensor("eye", [128, 128], MMDT, kind="ExternalInput"),
        "colw": nc.dram_tensor("colw", [128, 1 + BL * SC], MMDT,
                               kind="ExternalInput"),
        "consts": nc.dram_tensor("consts", [128, C_NCOL], F32, kind="ExternalInput"),
        "out_fp": nc.dram_tensor("out_fp", [BL, MEM], F32, kind="ExternalOutput"),
        "out_fc": nc.dram_tensor("out_fc", [BL, MEM], F32, kind="ExternalOutput"),
    }
    for br in ("fp", "fc"):
        for wn in ("wq", "wk", "w0", "w1"):
            name = f"{wn}T_{br}"
            dram[name] = nc.dram_tensor(name, [D, D], MMDT, kind="ExternalInput")
    if use_mask:
        dram["colmask"] = nc.dram_tensor("colmask", [BL, S], F32, kind="ExternalInput")
    if ln_affine:
        dram["a_bc"] = nc.dram_tensor("a_bc", [128, D], F32, kind="ExternalInput")
        dram["b_bc"] = nc.dram_tensor("b_bc", [128, D], F32, kind="ExternalInput")
    if gcn_bias:
        for br in ("fp", "fc"):
            for li in (0, 1):
                name = f"b{li}_bc_{br}"
                dram[name] = nc.dram_tensor(name, [128, MEM], F32, kind="ExternalInput")

    from contextlib import ExitStack
    with tile.TileContext(nc) as tc:
        with ExitStack() as ctx:
            _emit(nc, tc, dram, flags, ctx, n_iter=n_iter)
    nc.compile()
    return nc


class _Runner:
    """Cached jit(shard_map(bass_exec)) over the 8-core mesh.

    Mirrors bass2jax.run_bass_via_pjrt's multi-core path, but keeps the
    jitted executable and lets callers pre-stage inputs on device so
    repeated runs measure dispatch+execution, not host transfers.
    """

    def __init__(self, nc):
        import jax
        import concourse.mybir as mb
        from concourse import bass2jax
        from jax.experimental.shard_map import shard_map
        from jax.sharding import Mesh, NamedSharding, PartitionSpec

        bass2jax.install_neuronx_cc_hook()
        self.nc = nc
        partition_name = (nc.partition_id_tensor.name
                          if nc.partition_id_tensor else None)
        in_names, out_names, out_avals, zero_outs = [], [], [], []
        for alloc in nc.m.functions[0].allocations:
            if not isinstance(alloc, mb.MemoryLocationSet):
                continue
            name = alloc.memorylocations[0].name
            if alloc.kind == "ExternalInput":
                if name != partition_name:
                    in_names.append(name)
            elif alloc.kind == "ExternalOutput":
                shape = tuple(alloc.tensor_shape)
                dtype = mb.dt.np(alloc.dtype)
                out_names.append(name)
                out_avals.append(jax.core.ShapedArray(shape, dtype))
                zero_outs.append(np.zeros(shape, dtype))
        self.in_names = list(in_names)
        self.out_names = list(out_names)
        self.zero_outs = zero_outs
        n_params = len(in_names)
        n_outs = len(out_names)
        all_in_names = in_names + out_names
        if partition_name is not None:
            all_in_names = all_in_names + [partition_name]

        def _body(*args):
            operands = list(args)
            if partition_name is not None:
                operands.append(bass2jax.partition_id_tensor())
            outs = bass2jax._bass_exec_p.bind(
                *operands,
                out_avals=tuple(out_avals),
                in_names=tuple(all_in_names),
                out_names=tuple(out_names),
                lowering_input_output_aliases=(),
                sim_require_finite=True,
                sim_require_nnan=True,
                nc=nc,
            )
            return tuple(outs)

        devices = jax.devices()[:NCORES]
        self.mesh = Mesh(np.asarray(devices), ("core",))
        self.sharding = NamedSharding(self.mesh, PartitionSpec("core"))
        in_specs = (PartitionSpec("core"),) * (n_params + n_outs)
        out_specs = (PartitionSpec("core"),) * n_outs
        self.sharded = jax.jit(
            shard_map(_body, mesh=self.mesh, in_specs=in_specs,
                      out_specs=out_specs, check_rep=False),
            donate_argnums=tuple(range(n_params, n_params + n_outs)),
            keep_unused=True,
        )
        self._jax = jax

    def stage(self, in_maps):
        jax = self._jax
        concat = [
            np.concatenate([np.asarray(m[name]) for m in in_maps], axis=0)
            for name in self.in_names
        ]
        staged = jax.device_put(concat, [self.sharding] * len(concat))
        jax.block_until_ready(staged)
        return staged

    def run(self, staged):
        jax = self._jax
        zeros = jax.device_put(
            [np.zeros((NCORES * z.shape[0], *z.shape[1:]), z.dtype)
             for z in self.zero_outs],
            [self.sharding] * len(self.zero_outs))
        jax.block_until_ready(zeros)
        t0 = time.perf_counter()
        outs = self.sharded(*staged, *zeros)
        jax.block_until_ready(outs)
        dt_ns = (time.perf_counter() - t0) * 1e9
        results = [
            {name: np.asarray(outs[i]).reshape(NCORES, *self.zero_outs[i].shape)[c]
             for i, name in enumerate(self.out_names)}
            for c in range(NCORES)
        ]
        return results, dt_ns


_CACHE = {}
LAST_RUN_NS = None


def _get_program(flags):
    if flags not in _CACHE:
        nc = _build(flags)
        _CACHE[flags] = (nc, _Runner(nc))
    return _CACHE[flags]


def _make_in_maps(flags, a):
    use_mask, ln_affine, gcn_bias, qk_bias = flags
    f32 = np.float32
    eye = np.eye(128, dtype=f32)
    wTs = {}
    for br in ("fp", "fc"):
        for wn, key in (("wq", "Wq"), ("wk", "Wk"), ("w0", "W0"), ("w1", "W1")):
            wTs[f"{wn}T_{br}"] = np.ascontiguousarray(a[f"{br}_{key}"].T.astype(f32))

    def hcol(b):  # [D] -> [128, 8] head-major columns (96 used rows)
        out = np.zeros((128, 8), f32)
        out[:DK, :] = b.astype(f32).reshape(H, DK).T
        return out

    in_maps = []
    for ci in range(NCORES):
        sl = slice(ci * BL, (ci + 1) * BL)
        consts = np.zeros((128, C_NCOL), f32)
        consts[:, C_BQ["fp"]:C_BQ["fp"] + 8] = hcol(a["fp_bq"])
        consts[:, C_BK["fp"]:C_BK["fp"] + 8] = hcol(a["fp_bk"])
        consts[:, C_BQ["fc"]:C_BQ["fc"] + 8] = hcol(a["fc_bq"])
        consts[:, C_BK["fc"]:C_BK["fc"] + 8] = hcol(a["fc_bk"])
        asp = a["aspect_mask"][sl].astype(f32)          # [BL, S]
        consts[:, C_ASP:C_ASP + BL * SC] = asp.reshape(BL * SC, 128).T
        rm = (a["src_mask"][sl] != 0).astype(f32)       # [BL, S]
        consts[:, C_RM:C_RM + BL * SC] = rm.reshape(BL * SC, 128).T
        consts[:, C_ONE] = 1.0
        colw = np.zeros((128, 1 + BL * SC), f32)
        colw[:, 0] = 1.0
        colw[:, 1:] = asp.reshape(BL * SC, 128).T
        m = {
            "seq": np.ascontiguousarray(a["sequence_output"][sl].astype(f32)),
            "eye": eye,
            "consts": consts,
            "colw": colw,
        }
        m.update(wTs)
        if use_mask:
            m["colmask"] = rm.copy()
        if ln_affine:
            m["a_bc"] = np.broadcast_to(a["ln_a"].astype(f32), (128, D)).copy()
            m["b_bc"] = np.broadcast_to(a["ln_b"].astype(f32), (128, D)).copy()
        if gcn_bias:
            for br in ("fp", "fc"):
                for li in (0, 1):
                    m[f"b{li}_bc_{br}"] = np.broadcast_to(
                        a[f"{br}_b{li}"].astype(f32), (128, MEM)).copy()
        in_maps.append(m)
    return in_maps


def _host_tail(a, out_fp, out_fc):
    f = np.float64
    asp_wn = a["aspect_mask"].astype(f).sum(1)[:, None]
    outputs_fp = out_fp.astype(f)
    outputs_fc = out_fc.astype(f) / asp_wn

    def proj(x, y):
        y_mo = np.sqrt((y * y).sum(-1))
        xy = (x * y).sum(-1)
        yn = y / np.maximum(np.sqrt((y * y).sum(-1, keepdims=True)), 1e-12)
        return (xy / y_mo)[:, None] * yn

    fp_x = proj(outputs_fp, outputs_fc)
    fp_y = proj(outputs_fp, outputs_fp - fp_x)
    fc_y = outputs_fc
    logits_p = fp_y @ a["fp_dense_W"].astype(f).T + a["fp_dense_b"].astype(f)
    logits_c = fc_y @ a["fc_dense_W"].astype(f).T + a["fc_dense_b"].astype(f)
    return (logits_p.astype(np.float32), logits_c.astype(np.float32),
            fp_y.astype(np.float32),
            np.asarray(a["pooled_output"], dtype=np.float32))


def kernel(**inputs):
    global LAST_RUN_NS
    a = {k: np.asarray(v) for k, v in inputs.items()}
    flags = (
        bool(not np.all(a["src_mask"] == 1)),
        bool(not (np.all(a["ln_a"] == 1.0) and np.all(a["ln_b"] == 0.0))),
        bool(any(np.any(a[k] != 0.0) for k in
                 ("fp_b0", "fp_b1", "fc_b0", "fc_b1"))),
        bool(any(np.any(a[k] != 0.0) for k in
                 ("fp_bq", "fp_bk", "fc_bq", "fc_bk"))),
    )
    nc, runner = _get_program(flags)
    in_maps = _make_in_maps(flags, a)
    staged = runner.stage(in_maps)
    results, LAST_RUN_NS = runner.run(staged)
    out_fp = np.concatenate([results[ci]["out_fp"] for ci in range(NCORES)], axis=0)
    out_fc = np.concatenate([results[ci]["out_fc"] for ci in range(NCORES)], axis=0)
    return _host_tail(a, out_fp, out_fc)


# revision 34
# speedup vs baseline: 1.0345x; 1.0345x over previous
"""Trainium2 Bass kernel for nn_DACA_29343216566277 (dual-GCN message passing).

Data-parallel over batch: 8 NeuronCores x 4 examples each. The full
per-example pipeline (LayerNorm -> attention-softmax adjacency -> 2-layer
GCN, two branches) runs on-device; the tiny [B,768]-level projection /
dense tail runs on host numpy.

Self-contained: hardcodes shapes/sharding; only imports the concourse
framework from the container's /opt install.
"""

import os
import sys
import time

import numpy as np

for _p in ("/opt/trn_rl_repo", "/root/.axon_site/_ro/trn_rl_repo"):
    if os.path.isdir(_p) and _p not in sys.path:
        sys.path.insert(0, _p)

import concourse.bass as bass
import concourse.bacc as bacc
import concourse.mybir as mybir
import concourse.tile as tile
from concourse.bass_utils import run_bass_kernel_spmd

AF = mybir.ActivationFunctionType
ALU = mybir.AluOpType
AX = mybir.AxisListType
F32 = mybir.dt.float32
PSUM = bass.MemorySpace.PSUM
DRAM = bass.MemorySpace.DRAM

B, S, D, H, MEM = 32, 512, 768, 8, 768
DK = D // H          # 96
NCORES = 8
BL = B // NCORES     # 4 examples per core
SC = S // 128        # 4 s-chunks
DC = D // 128        # 6 d-chunks
LN_EPS = 1e-6
INV_SQRT_DK = float(1.0 / np.sqrt(np.float32(DK)))

# PE dtype for matmuls: float32r streams 1 row/cycle (vs 4 for fp32) when the
# moving free dim is >=256; same 4-byte layout, relaxed multiply precision.
F32R = mybir.dt.float32r
USE_F32R = os.environ.get("KERNEL_MM_F32R", "1") == "1"
MMDT = F32R if USE_F32R else F32


def _r(ap):
    return ap

# consts tile column layout
C_BQ = {"fp": 0, "fc": 16}
C_BK = {"fp": 8, "fc": 24}
C_ASP = 32           # + ex*SC + sc : aspect-mask column chunks
C_RM = 48            # + ex*SC + sc : row (src) mask column chunks
C_ONE = 64
C_ZERO = 65
C_NCOL = 72


def _emit(nc, tc, dram, flags, ctx, n_iter=1):
    use_mask, ln_affine, gcn_bias, qk_bias = flags
    sb = ctx.enter_context(tc.tile_pool(name="sb", bufs=1))
    ps = ctx.enter_context(tc.tile_pool(name="ps", bufs=1, space=PSUM))
    dp = ctx.enter_context(tc.tile_pool(name="dp", bufs=1, space=DRAM))

    eye = sb.tile([128, 128], MMDT, name="eye_sb", tag="eye")
    nc.sync.dma_start(eye, dram["eye"].ap())
    omi = sb.tile([128, 128], F32, name="omi_sb", tag="omi")
    nc.vector.tensor_scalar(out=omi, in0=eye, scalar1=-1.0, scalar2=1.0,
                            op0=ALU.mult, op1=ALU.add)
    cst = sb.tile([128, C_NCOL], F32, name="cst_sb", tag="cst")
    nc.sync.dma_start(cst, dram["consts"].ap())
    colw = sb.tile([128, 1 + BL * SC], MMDT, name="colw_sb", tag="colw")
    nc.sync.dma_start(colw, dram["colw"].ap())
    zc = cst[:, C_ZERO:C_ZERO + 1]

    if ln_affine:
        abc = sb.tile([128, D], F32, name="abc_sb", tag="abc")
        nc.sync.dma_start(abc, dram["a_bc"].ap())
        bbc = sb.tile([128, D], F32, name="bbc_sb", tag="bbc")
        nc.sync.dma_start(bbc, dram["b_bc"].ap())
    if gcn_bias:
        gb = {}
        for br in ("fp", "fc"):
            for li in (0, 1):
                t = sb.tile([128, MEM], F32, name=f"b{li}bc_{br}", tag=f"b{li}bc_{br}")
                nc.sync.dma_start(t, dram[f"b{li}_bc_{br}"].ap())
                gb[(br, li)] = t
    if use_mask:
        cms = []
        for ex in range(BL):
            cmrow = sb.tile([1, S], F32, name=f"cmrow{ex}", tag="cmrow", bufs=2)
            nc.sync.dma_start(cmrow, dram["colmask"].ap()[ex:ex + 1, :])
            cm = sb.tile([128, S], F32, name=f"cm{ex}", tag=f"cm{ex}")
            nc.gpsimd.partition_broadcast(cm, cmrow)
            cms.append(cm)

    seq = dram["seq"].ap()

    if n_iter > 1:
        ctx.enter_context(tc.For_i(0, n_iter, 1))

    # ---- Phase 0: LayerNorm + transpose, spilled to DRAM ----
    xd = [dp.tile([128, SC, D], MMDT, name=f"xd{ex}", tag=f"xd{ex}")
          for ex in range(BL)]
    xTd = [dp.tile([128, DC, S], MMDT, name=f"xTd{ex}", tag=f"xTd{ex}")
           for ex in range(BL)]

    def ln_block(ex):
        xr = sb.tile([128, SC, D], MMDT, name=f"xr{ex}", tag="x", bufs=2)
        nc.sync.dma_start(xr, seq[ex].rearrange("(c p) d -> p c d", p=128))
        mvs = sb.tile([128, SC, 2], F32, name=f"mvs{ex}", tag="mvs", bufs=2)
        for sc in range(SC):
            bnst = sb.tile([128, 2, 6], F32, name=f"bnst{ex}_{sc}", tag="bnst", bufs=2)
            nc.vector.bn_stats(bnst[:, 0, :], xr[:, sc, 0:384])
            nc.vector.bn_stats(bnst[:, 1, :], xr[:, sc, 384:768])
            nc.vector.bn_aggr(mvs[:, sc, :], bnst)
        # rln = 1 / (sqrt(var * N/(N-1)) + eps); sqrt via exp(0.5*ln(v)),
        # batched over all 4 s-chunks so Ln/Exp table sets load once per ex
        lnv = sb.tile([128, SC], F32, name=f"lnv{ex}", tag="lnv", bufs=2)
        nc.scalar.activation(lnv, mvs[:, :, 1], AF.Ln, bias=zc, scale=float(D / (D - 1)))
        sd = sb.tile([128, SC], F32, name=f"sd{ex}", tag="sd", bufs=2)
        nc.scalar.activation(sd, lnv, AF.Exp, bias=zc, scale=0.5)
        nc.vector.tensor_scalar(out=sd, in0=sd, scalar1=LN_EPS, scalar2=None, op0=ALU.add)
        rln = sb.tile([128, SC], F32, name=f"rln{ex}", tag="rln", bufs=2)
        nc.vector.reciprocal(rln, sd)
        nm = sb.tile([128, SC], F32, name=f"nm{ex}", tag="nm", bufs=2)
        nc.vector.tensor_tensor(out=nm, in0=mvs[:, :, 0], in1=rln, op=ALU.mult)
        nc.vector.tensor_scalar(out=nm, in0=nm, scalar1=-1.0, scalar2=None, op0=ALU.mult)
        for sc in range(SC):
            nc.vector.tensor_scalar(out=xr[:, sc, :], in0=xr[:, sc, :],
                                    scalar1=rln[:, sc:sc + 1], scalar2=nm[:, sc:sc + 1],
                                    op0=ALU.mult, op1=ALU.add)
            if ln_affine:
                nc.vector.tensor_tensor(out=xr[:, sc, :], in0=xr[:, sc, :], in1=abc, op=ALU.mult)
                nc.vector.tensor_tensor(out=xr[:, sc, :], in0=xr[:, sc, :], in1=bbc, op=ALU.add)
        nc.sync.dma_start(xd[ex][:, :, :], xr[:, :, :])
        xT0 = sb.tile([128, DC, S], MMDT, name=f"xT0_{ex}", tag="xT", bufs=2)
        for dc in range(DC):
            tps = ps.tile([128, 512], MMDT, name=f"tp0_{ex}_{dc}", tag="ps", bufs=8)
            for sc in range(SC):
                nc.tensor.transpose(_r(tps[:, sc * 128:(sc + 1) * 128]),
                                    _r(xr[:, sc, dc * 128:(dc + 1) * 128]), _r(eye))
            nc.vector.tensor_copy(xT0[:, dc, :], tps)
        nc.sync.dma_start(xTd[ex][:, :, :], xT0[:, :, :])
        return xr, xT0

    # ---- Branches ----
    for br in ("fp", "fc"):
        ws = {}
        for wn in ("wq", "wk", "w0", "w1"):
            t = sb.tile([128, DC, D], MMDT, name=f"{wn}_{br}", tag=wn)
            nc.sync.dma_start(t, dram[f"{wn}T_{br}"].ap().rearrange("(c p) d -> p c d", p=128))
            ws[wn] = t
        for ex in range(BL):
            if br == "fp":
                x_sb, xT = ln_block(ex)
            else:
                x_sb = sb.tile([128, SC, D], MMDT, name=f"x_{br}{ex}", tag="x", bufs=2)
                nc.sync.dma_start(x_sb, xd[ex][:, :, :])
                xT = sb.tile([128, DC, S], MMDT, name=f"xT_{br}{ex}", tag="xT", bufs=2)
                nc.sync.dma_start(xT, xTd[ex][:, :, :])

            # --- scores / softmax / adjacency, head-major ---
            adj_ps = [ps.tile([128, 512], F32, name=f"adjps_{br}{ex}m{m}",
                              tag="ps", bufs=8) for m in range(SC)]
            for h in range(H):
                qph = ps.tile([96, 512], F32, name=f"qph_{br}{ex}h{h}", tag="ps", bufs=8)
                for kc in range(DC):
                    nc.tensor.matmul(qph, _r(ws["wq"][:, kc, DK * h:DK * (h + 1)]),
                                     _r(xT[:, kc, :]), start=(kc == 0), stop=(kc == DC - 1))
                qTh = sb.tile([96, 512], MMDT, name=f"qT_{br}{ex}h{h}", tag="qT", bufs=2)
                if qk_bias:
                    nc.vector.tensor_scalar(out=qTh, in0=qph,
                                            scalar1=cst[:96, C_BQ[br] + h:C_BQ[br] + h + 1],
                                            scalar2=None, op0=ALU.add)
                else:
                    nc.scalar.copy(qTh, qph)
                kph = ps.tile([96, 512], F32, name=f"kph_{br}{ex}h{h}", tag="ps", bufs=8)
                for kc in range(DC):
                    nc.tensor.matmul(kph, _r(ws["wk"][:, kc, DK * h:DK * (h + 1)]),
                                     _r(xT[:, kc, :]), start=(kc == 0), stop=(kc == DC - 1))
                kTh = sb.tile([96, 512], MMDT, name=f"kT_{br}{ex}h{h}", tag="kT", bufs=2)
                if qk_bias:
                    nc.vector.tensor_scalar(out=kTh, in0=kph,
                                            scalar1=cst[:96, C_BK[br] + h:C_BK[br] + h + 1],
                                            scalar2=None, op0=ALU.add)
                else:
                    nc.vector.tensor_copy(kTh, kph)
                ehs = [sb.tile([128, 512], MMDT, name=f"e_{br}{ex}h{h}m{m}",
                               tag="e", bufs=4) for m in range(SC)]
                rs = sb.tile([128, SC], F32, name=f"rs_{br}{ex}h{h}", tag="rs", bufs=2)
                for m in range(SC):
                    sps = ps.tile([128, 512], F32, name=f"sps_{br}{ex}h{h}m{m}",
                                  tag="ps", bufs=8)
                    nc.tensor.matmul(sps, _r(qTh[:, m * 128:(m + 1) * 128]), _r(kTh[:, :]),
                                     start=True, stop=True)
                    if use_mask:
                        nc.scalar.activation(ehs[m], sps, AF.Exp, bias=zc,
                                             scale=INV_SQRT_DK)
                        nc.vector.tensor_tensor_reduce(
                            out=ehs[m], in0=ehs[m], in1=cms[ex], scale=1.0,
                            scalar=0.0, op0=ALU.mult, op1=ALU.add,
                            accum_out=rs[:, m:m + 1])
                    else:
                        nc.scalar.activation(ehs[m], sps, AF.Exp, bias=zc,
                                             scale=INV_SQRT_DK, accum_out=rs[:, m:m + 1])
                rra = sb.tile([128, SC], F32, name=f"rra_{br}{ex}h{h}", tag="rra", bufs=2)
                nc.vector.tensor_scalar(out=rra, in0=rs, scalar1=1e-30, scalar2=None,
                                        op0=ALU.add)
                rr = sb.tile([128, SC], F32, name=f"rr_{br}{ex}h{h}", tag="rr", bufs=2)
                nc.vector.reciprocal(rr, rra)
                for m in range(SC):
                    dg = sb.tile([128, 128], MMDT, name=f"dg_{br}{ex}h{h}m{m}",
                                 tag="dg", bufs=2)
                    nc.gpsimd.tensor_scalar(out=dg, in0=eye, scalar1=rr[:, m:m + 1],
                                            scalar2=1.0 / H, op0=ALU.mult, op1=ALU.mult)
                    nc.tensor.matmul(adj_ps[m], _r(dg), _r(ehs[m]),
                                     start=(h == 0), stop=(h == H - 1),
                                     skip_group_check=True)

            adj_sb = sb.tile([128, SC, S], MMDT, name=f"adj_{br}{ex}", tag="adj")
            dsum = sb.tile([128, SC], F32, name=f"dsum_{br}{ex}", tag="dsum", bufs=2)
            for m in range(SC):
                if use_mask:
                    rmc = cst[:, C_RM + ex * SC + m:C_RM + ex * SC + m + 1]
                    nc.vector.tensor_scalar(out=adj_sb[:, m, :], in0=adj_ps[m],
                                            scalar1=rmc, scalar2=None, op0=ALU.mult)
                    me = sb.tile([128, 128], F32, name=f"me_{br}{ex}m{m}", tag="me", bufs=2)
                    nc.vector.tensor_scalar(out=me, in0=eye, scalar1=rmc, scalar2=None,
                                            op0=ALU.mult)
                    blk = adj_sb[:, m, m * 128:(m + 1) * 128]
                    nc.vector.tensor_tensor(out=blk, in0=blk, in1=omi, op=ALU.mult)
                    nc.vector.tensor_tensor(out=blk, in0=blk, in1=me, op=ALU.add)
                else:
                    nc.vector.tensor_copy(adj_sb[:, m, :], adj_ps[m])
                    blk = adj_sb[:, m, m * 128:(m + 1) * 128]
                    nc.gpsimd.tensor_tensor(out=blk, in0=blk, in1=omi, op=ALU.mult)
                    nc.gpsimd.tensor_tensor(out=blk, in0=blk, in1=eye, op=ALU.add)
                nc.vector.reduce_sum(out=dsum[:, m:m + 1], in_=adj_sb[:, m, :], axis=AX.X)
            dtm = sb.tile([128, SC], F32, name=f"dtm_{br}{ex}", tag="dtm", bufs=2)
            nc.vector.tensor_scalar(out=dtm, in0=dsum, scalar1=1.0, scalar2=None, op0=ALU.add)
            drp = sb.tile([128, SC], F32, name=f"drp_{br}{ex}", tag="drp", bufs=2)
            nc.vector.reciprocal(drp, dtm)

            adjT = sb.tile([128, SC, S], MMDT, name=f"adjT_{br}{ex}", tag="adjT", bufs=1)
            for b in range(SC):
                tps = ps.tile([128, 512], MMDT, name=f"tpa_{br}{ex}b{b}", tag="ps", bufs=8)
                for a in range(SC):
                    nc.tensor.transpose(_r(tps[:, a * 128:(a + 1) * 128]),
                                        _r(adj_sb[:, a, b * 128:(b + 1) * 128]), _r(eye))
                nc.vector.tensor_copy(adjT[:, b, :], tps)

            # --- GCN layer 1 ---
            t1 = sb.tile([128, DC, S], MMDT, name=f"t1_{br}{ex}", tag="tT")
            for dc in range(DC):
                tps = ps.tile([128, 512], F32, name=f"tl1_{br}{ex}d{dc}", tag="ps", bufs=8)
                for jc in range(SC):
                    nc.tensor.matmul(tps, _r(x_sb[:, jc, dc * 128:(dc + 1) * 128]),
                                     _r(adjT[:, jc, :]), start=(jc == 0), stop=(jc == SC - 1))
                nc.vector.tensor_copy(t1[:, dc, :], tps)
            o1 = sb.tile([128, SC, MEM], MMDT, name=f"o1_{br}{ex}", tag="o1")
            for sc in range(SC):
                up1 = ps.tile([128, 512], F32, name=f"u1a_{br}{ex}s{sc}", tag="ps", bufs=8)
                up2 = ps.tile([128, 256], F32, name=f"u1b_{br}{ex}s{sc}", tag="ps", bufs=8)
                for dc in range(DC):
                    lh = _r(t1[:, dc, sc * 128:(sc + 1) * 128])
                    nc.tensor.matmul(up1, lh, _r(ws["w0"][:, dc, 0:512]),
                                     start=(dc == 0), stop=(dc == DC - 1))
                    nc.tensor.matmul(up2, lh, _r(ws["w0"][:, dc, 512:MEM]),
                                     start=(dc == 0), stop=(dc == DC - 1))
                for up, c0, c1 in ((up1, 0, 512), (up2, 512, MEM)):
                    if gcn_bias:
                        nc.vector.tensor_tensor(out=o1[:, sc, c0:c1], in0=up,
                                                in1=gb[(br, 0)][:, c0:c1], op=ALU.add)
                        nc.vector.tensor_scalar(out=o1[:, sc, c0:c1], in0=o1[:, sc, c0:c1],
                                                scalar1=drp[:, sc:sc + 1], scalar2=0.0,
                                                op0=ALU.mult, op1=ALU.max)
                    else:
                        nc.vector.tensor_scalar(out=o1[:, sc, c0:c1], in0=up,
                                                scalar1=drp[:, sc:sc + 1], scalar2=0.0,
                                                op0=ALU.mult, op1=ALU.max)

            # --- GCN layer 2 ---
            t2 = sb.tile([128, DC, S], MMDT, name=f"t2_{br}{ex}", tag="tT")
            for mc in range(DC):
                tps = ps.tile([128, 512], F32, name=f"tl2_{br}{ex}d{mc}", tag="ps", bufs=8)
                for jc in range(SC):
                    nc.tensor.matmul(tps, _r(o1[:, jc, mc * 128:(mc + 1) * 128]),
                                     _r(adjT[:, jc, :]), start=(jc == 0), stop=(jc == SC - 1))
                nc.vector.tensor_copy(t2[:, mc, :], tps)
            o2 = sb.tile([128, SC, MEM], MMDT, name=f"o2_{br}{ex}", tag="o2")
            for sc in range(SC):
                up1 = ps.tile([128, 512], F32, name=f"u2a_{br}{ex}s{sc}", tag="ps", bufs=8)
                up2 = ps.tile([128, 256], F32, name=f"u2b_{br}{ex}s{sc}", tag="ps", bufs=8)
                for mc in range(DC):
                    lh = _r(t2[:, mc, sc * 128:(sc + 1) * 128])
                    nc.tensor.matmul(up1, lh, _r(ws["w1"][:, mc, 0:512]),
                                     start=(mc == 0), stop=(mc == DC - 1))
                    nc.tensor.matmul(up2, lh, _r(ws["w1"][:, mc, 512:MEM]),
                                     start=(mc == 0), stop=(mc == DC - 1))
                for up, c0, c1 in ((up1, 0, 512), (up2, 512, MEM)):
                    if gcn_bias:
                        nc.vector.tensor_tensor(out=o2[:, sc, c0:c1], in0=up,
                                                in1=gb[(br, 1)][:, c0:c1], op=ALU.add)
                        nc.vector.tensor_scalar(out=o2[:, sc, c0:c1], in0=o2[:, sc, c0:c1],
                                                scalar1=drp[:, sc:sc + 1], scalar2=0.0,
                                                op0=ALU.mult, op1=ALU.max)
                    else:
                        nc.vector.tensor_scalar(out=o2[:, sc, c0:c1], in0=up,
                                                scalar1=drp[:, sc:sc + 1], scalar2=0.0,
                                                op0=ALU.mult, op1=ALU.max)

            # --- weighted column-sum over S ---
            cs1 = ps.tile([1, 512], F32, name=f"cs1_{br}{ex}", tag="ps", bufs=8)
            cs2 = ps.tile([1, 256], F32, name=f"cs2_{br}{ex}", tag="ps", bufs=8)
            for sc in range(SC):
                if br == "fp":
                    colv = colw[:, 0:1]
                else:
                    colv = colw[:, 1 + ex * SC + sc:2 + ex * SC + sc]
                nc.tensor.matmul(cs1[0:1, :], _r(colv), _r(o2[:, sc, 0:512]),
                                 start=(sc == 0), stop=(sc == SC - 1),
                                 skip_group_check=True)
                nc.tensor.matmul(cs2[0:1, :], _r(colv), _r(o2[:, sc, 512:MEM]),
                                 start=(sc == 0), stop=(sc == SC - 1),
                                 skip_group_check=True)
            stg = sb.tile([1, MEM], F32, name=f"stg_{br}{ex}", tag="stg", bufs=1)
            nc.vector.tensor_copy(stg[:, 0:512], cs1)
            nc.vector.tensor_copy(stg[:, 512:MEM], cs2)
            nc.sync.dma_start(dram[f"out_{br}"].ap()[ex:ex + 1, :], stg)


def _build(flags, n_iter=1):
    use_mask, ln_affine, gcn_bias, qk_bias = flags
    nc = bacc.Bacc("TRN2", target_bir_lowering=False, debug=False,
                   num_devices=NCORES)
    dram = {
        "seq": nc.dram_tensor("seq", [BL, S, D], MMDT, kind="ExternalInput"),
        "eye": nc.dram_tensor("eye", [128, 128], MMDT, kind="ExternalInput"),
        "colw": nc.dram_tensor("colw", [128, 1 + BL * SC], MMDT,
                               kind="ExternalInput"),
        "consts": nc.dram_tensor("consts", [128, C_NCOL], F32, kind="ExternalInput"),
        "out_fp": nc.dram_tensor("out_fp", [BL, MEM], F32, kind="ExternalOutput"),
        "out_fc": nc.dram_tensor("out_fc", [BL, MEM], F32, kind="ExternalOutput"),
    }
    for br in ("fp", "fc"):
        for wn in ("wq", "wk", "w0", "w1"):
            name = f"{wn}T_{br}"
            dram[name] = nc.dram_tensor(name, [D, D], MMDT, kind="ExternalInput")
    if use_mask:
        dram["colmask"] = nc.dram_tensor("colmask", [BL, S], F32, kind="ExternalInput")
    if ln_affine:
        dram["a_bc"] = nc.dram_tensor("a_bc", [128, D], F32, kind="ExternalInput")
        dram["b_bc"] = nc.dram_tensor("b_bc", [128, D], F32, kind="ExternalInput")
    if gcn_bias:
        for br in ("fp", "fc"):
            for li in (0, 1):
                name = f"b{li}_bc_{br}"
                dram[name] = nc.dram_tensor(name, [128, MEM], F32, kind="ExternalInput")

    from contextlib import ExitStack
    with tile.TileContext(nc) as tc:
        with ExitStack() as ctx:
            _emit(nc, tc, dram, flags, ctx, n_iter=n_iter)
    nc.compile()
    return nc


class _Runner:
    """Cached jit(shard_map(bass_exec)) over the 8-core mesh.

    Mirrors bass2jax.run_bass_via_pjrt's multi-core path, but keeps the
    jitted executable and lets callers pre-stage inputs on device so
    repeated runs measure dispatch+execution, not host transfers.
    """

    def __init__(self, nc):
        import jax
        import concourse.mybir as mb
        from concourse import bass2jax
        from jax.experimental.shard_map import shard_map
        from jax.sharding import Mesh, NamedSharding, PartitionSpec

        bass2jax.install_neuronx_cc_hook()
        self.nc = nc
        partition_name = (nc.partition_id_tensor.name
                          if nc.partition_id_tensor else None)
        in_names, out_names, out_avals, zero_outs = [], [], [], []
        for alloc in nc.m.functions[0].allocations:
            if not isinstance(alloc, mb.MemoryLocationSet):
                continue
            name = alloc.memorylocations[0].name
            if alloc.kind == "ExternalInput":
                if name != partition_name:
                    in_names.append(name)
            elif alloc.kind == "ExternalOutput":
                shape = tuple(alloc.tensor_shape)
                dtype = mb.dt.np(alloc.dtype)
                out_names.append(name)
                out_avals.append(jax.core.ShapedArray(shape, dtype))
                zero_outs.append(np.zeros(shape, dtype))
        self.in_names = list(in_names)
        self.out_names = list(out_names)
        self.zero_outs = zero_outs
        n_params = len(in_names)
        n_outs = len(out_names)
        all_in_names = in_names + out_names
        if partition_name is not None:
            all_in_names = all_in_names + [partition_name]

        def _body(*args):
            operands = list(args)
            if partition_name is not None:
                operands.append(bass2jax.partition_id_tensor())
            outs = bass2jax._bass_exec_p.bind(
                *operands,
                out_avals=tuple(out_avals),
                in_names=tuple(all_in_names),
                out_names=tuple(out_names),
                lowering_input_output_aliases=(),
                sim_require_finite=True,
                sim_require_nnan=True,
                nc=nc,
            )
            return tuple(outs)

        devices = jax.devices()[:NCORES]
        self.mesh = Mesh(np.asarray(devices), ("core",))
        self.sharding = NamedSharding(self.mesh, PartitionSpec("core"))
        in_specs = (PartitionSpec("core"),) * (n_params + n_outs)
        out_specs = (PartitionSpec("core"),) * n_outs
        self.sharded = jax.jit(
            shard_map(_body, mesh=self.mesh, in_specs=in_specs,
                      out_specs=out_specs, check_rep=False),
            donate_argnums=tuple(range(n_params, n_params + n_outs)),
            keep_unused=True,
        )
        self._jax = jax

    def stage(self, in_maps):
        jax = self._jax
        concat = [
            np.concatenate([np.asarray(m[name]) for m in in_maps], axis=0)
            for name in self.in_names
        ]
        staged = jax.device_put(concat, [self.sharding] * len(concat))
        jax.block_until_ready(staged)
        return staged

    def run(self, staged):
        jax = self._jax
        zeros = jax.device_put(
            [np.zeros((NCORES * z.shape[0], *z.shape[1:]), z.dtype)
             for z in self.zero_outs],
            [self.sharding] * len(self.zero_outs))
        jax.block_until_ready(zeros)
        t0 = time.perf_counter()
        outs = self.sharded(*staged, *zeros)
        jax.block_until_ready(outs)
        dt_ns = (time.perf_counter() - t0) * 1e9
        results = [
            {name: np.asarray(outs[i]).reshape(NCORES, *self.zero_outs[i].shape)[c]
             for i, name in enumerate(self.out_names)}
            for c in range(NCORES)
        ]
        return results, dt_ns


_CACHE = {}
LAST_RUN_NS = None


def _get_program(flags):
    if flags not in _CACHE:
        nc = _build(flags)
        _CACHE[flags] = (nc, _Runner(nc))
    return _CACHE[flags]


def _make_in_maps(flags, a):
    use_mask, ln_affine, gcn_bias, qk_bias = flags
    f32 = np.float32
    eye = np.eye(128, dtype=f32)
    wTs = {}
    for br in ("fp", "fc"):
        for wn, key in (("wq", "Wq"), ("wk", "Wk"), ("w0", "W0"), ("w1", "W1")):
            wTs[f"{wn}T_{br}"] = np.ascontiguousarray(a[f"{br}_{key}"].T.astype(f32))

    def hcol(b):  # [D] -> [128, 8] head-major columns (96 used rows)
        out = np.zeros((128, 8), f32)
        out[:DK, :] = b.astype(f32).reshape(H, DK).T
        return out

    in_maps = []
    for ci in range(NCORES):
        sl = slice(ci * BL, (ci + 1) * BL)
        consts = np.zeros((128, C_NCOL), f32)
        consts[:, C_BQ["fp"]:C_BQ["fp"] + 8] = hcol(a["fp_bq"])
        consts[:, C_BK["fp"]:C_BK["fp"] + 8] = hcol(a["fp_bk"])
        consts[:, C_BQ["fc"]:C_BQ["fc"] + 8] = hcol(a["fc_bq"])
        consts[:, C_BK["fc"]:C_BK["fc"] + 8] = hcol(a["fc_bk"])
        asp = a["aspect_mask"][sl].astype(f32)          # [BL, S]
        consts[:, C_ASP:C_ASP + BL * SC] = asp.reshape(BL * SC, 128).T
        rm = (a["src_mask"][sl] != 0).astype(f32)       # [BL, S]
        consts[:, C_RM:C_RM + BL * SC] = rm.reshape(BL * SC, 128).T
        consts[:, C_ONE] = 1.0
        colw = np.zeros((128, 1 + BL * SC), f32)
        colw[:, 0] = 1.0
        colw[:, 1:] = asp.reshape(BL * SC, 128).T
        m = {
            "seq": np.ascontiguousarray(a["sequence_output"][sl].astype(f32)),
            "eye": eye,
            "consts": consts,
            "colw": colw,
        }
        m.update(wTs)
        if use_mask:
            m["colmask"] = rm.copy()
        if ln_affine:
            m["a_bc"] = np.broadcast_to(a["ln_a"].astype(f32), (128, D)).copy()
            m["b_bc"] = np.broadcast_to(a["ln_b"].astype(f32), (128, D)).copy()
        if gcn_bias:
            for br in ("fp", "fc"):
                for li in (0, 1):
                    m[f"b{li}_bc_{br}"] = np.broadcast_to(
                        a[f"{br}_b{li}"].astype(f32), (128, MEM)).copy()
        in_maps.append(m)
    return in_maps


def _host_tail(a, out_fp, out_fc):
    f = np.float64
    asp_wn = a["aspect_mask"].astype(f).sum(1)[:, None]
    outputs_fp = out_fp.astype(f)
    outputs_fc = out_fc.astype(f) / asp_wn

    def proj(x, y):
        y_mo = np.sqrt((y * y).sum(-1))
        xy = (x * y).sum(-1)
        yn = y / np.maximum(np.sqrt((y * y).sum(-1, keepdims=True)), 1e-12)
        return (xy / y_mo)[:, None] * yn

    fp_x = proj(outputs_fp, outputs_fc)
    fp_y = proj(outputs_fp, outputs_fp - fp_x)
    fc_y = outputs_fc
    logits_p = fp_y @ a["fp_dense_W"].astype(f).T + a["fp_dense_b"].astype(f)
    logits_c = fc_y @ a["fc_dense_W"].astype(f).T + a["fc_dense_b"].astype(f)
    return (logits_p.astype(np.float32), logits_c.astype(np.float32),
            fp_y.astype(np.float32),
            np.asarray(a["pooled_output"], dtype=np.float32))


def kernel(**inputs):
    global LAST_RUN_NS
    a = {k: np.asarray(v) for k, v in inputs.items()}
    flags = (
        bool(not np.all(a["src_mask"] == 1)),
        bool(not (np.all(a["ln_a"] == 1.0) and np.all(a["ln_b"] == 0.0))),
        bool(any(np.any(a[k] != 0.0) for k in
                 ("fp_b0", "fp_b1", "fc_b0", "fc_b1"))),
        bool(any(np.any(a[k] != 0.0) for k in
                 ("fp_bq", "fp_bk", "fc_bq", "fc_bk"))),
    )
    nc, runner = _get_program(flags)
    in_maps = _make_in_maps(flags, a)
    staged = runner.stage(in_maps)
    results, LAST_RUN_NS = runner.run(staged)
    out_fp = np.concatenate([results[ci]["out_fp"] for ci in range(NCORES)], axis=0)
    out_fc = np.concatenate([results[ci]["out_fc"] for ci in range(NCORES)], axis=0)
    return _host_tail(a, out_fp, out_fc)
